# revision 16
# baseline (speedup 1.0000x reference)
"""Trainium2 Bass kernel for nn_DETRLoss.

Strategy (pure data parallel, batch dim N=8 over 8 NeuronCores):

The only memory-heavy input is img_features [8, 2048, 42, 42] (115.6 MB).
It feeds the loss ONLY through: channel-mean -> bilinear upsample to
(h, w) -> summed-area table -> per-query crop means -> top-5 *indices*.
The SAT of a bilinear upsample evaluated at integer pixel corners is a
bilinear form of the 42x42 channel-mean f:

    sat[y, x] = CA[y] @ f @ CB[x]^T

where CA/CB are cumulative-sum rows of the (analytic) resize matrices.
So each query's crop sum is (CA[y2]-CA[y1]) @ f @ (CB[x2]-CB[x1])^T:
no 1333x1333 upsample or SAT is ever materialized. The crop means feed
ONLY a top-5 selection, so small rounding differences are harmless.

Per core (one image): stream 2048x1764 features (14.45 MB); the channel
sum is computed with the FEATURE TILE AS THE STATIONARY matmul operand
(ones as the moving vector), so each 126-column block lands directly as
one PSUM column of f2 [126, 14] -- partition-major, no [1, 1764] row,
no reshape round-trip. The crop means follow from 3 masked matmuls
(126 = 3x42 row-blocks, mask baked into the host-built C weights), an
elementwise R-weight multiply, and a ones contraction. Top-5 via Max8 +
MatchReplace; all CE/BCE/L1/IoU terms on-chip with host-folded scale
coefficients; the last two tiles stream as 4 column-slivers each so the
tail chases the final DMA bytes. Output: per-image scalar loss; host
sums the 8 scalars.
"""

import ml_dtypes
import numpy as np

import bass_rust
import concourse.bass as bass
import concourse.mybir as mybir
from concourse.bass_utils import run_bass_kernel_spmd
from concourse.tile import TileContext

F32 = mybir.dt.float32
BF16 = mybir.dt.bfloat16
AF = mybir.ActivationFunctionType
ALU = mybir.AluOpType
AX = mybir.AxisListType

N, Q, CC = 8, 300, 92
CF, HF, WF = 2048, 42, 42
M, TOPK = 20, 5
NUM_CLASSES = 91
NEG = -1e11
QP = 384  # Q padded to 3*128
POS = HF * WF  # 1764
NREST = Q - M - TOPK  # 275: matched queries are unique, top-5 disjoint

# column chunks (sliver DMAs / fused-add chunks), aligned to 126-blocks
CHUNKS = [(0, 504, range(0, 4)), (504, 1008, range(4, 8)),
          (1008, 1386, range(8, 11)), (1386, 1764, range(11, 14))]


def _split_sync_waits(nc, max_waits=1):
    """This walrus build rejects >2 sync waits on one instruction ("Too
    many sync wait commands"); hoist extra waits onto same-engine nops
    emitted immediately before the instruction (identical semantics:
    engines process waits in program order)."""
    ctr = 0
    for f in nc.m.functions:
        for bb in f.blocks:
            out = []
            for inst in bb.instructions:
                si = inst.sync_info
                waits = list(si.on_wait) if si and si.on_wait else []
                if len(waits) > max_waits:
                    for w in waits[:-max_waits]:
                        ctr += 1
                        out.append(bass_rust.InstNoOp(
                            name=f"I-wsplit{ctr}", engine=inst.engine,
                            ins=[], outs=[],
                            sync_info=bass_rust.SyncInfo(
                                on_wait=[w], on_update=[])))
                    inst.sync_info = bass_rust.SyncInfo(
                        on_wait=waits[-max_waits:],
                        on_update=list(si.on_update or []))
                out.append(inst)
            bb.instructions = out


# ---------------------------------------------------------------- host prep

def _interp_cummat(out_size, in_size):
    """CA [out_size+1, in_size] with CA[y] = sum_{i<y} A[i,:], A the
    half-pixel-centered bilinear resize matrix (jax.image.resize)."""
    A = np.zeros((out_size, in_size), np.float64)
    scale = in_size / out_size
    for i in range(out_size):
        src = (i + 0.5) * scale - 0.5
        i0 = int(np.floor(src))
        w1 = src - i0
        j0 = min(max(i0, 0), in_size - 1)
        j1 = min(max(i0 + 1, 0), in_size - 1)
        A[i, j0] += 1.0 - w1
        A[i, j1] += w1
    CA = np.zeros((out_size + 1, in_size), np.float64)
    np.cumsum(A, 0, out=CA[1:])
    return CA.astype(np.float32)


def _prep_core(n, pred_logits, pred_boxes, tgt_labels, tgt_boxes,
               query_idx, tgt_idx, h, w, CAh, CBw):
    """Build the small per-core input tensors (everything except feat)."""
    scale = np.array([w, h, w, h], np.float32)
    pb = pred_boxes[n].astype(np.float32)  # [300,4]
    cx, cy, bw, bh = pb[:, 0], pb[:, 1], pb[:, 2], pb[:, 3]
    xy = np.stack([cx - bw / 2, cy - bh / 2, cx + bw / 2, cy + bh / 2], -1)
    bb = xy * scale
    x1 = np.clip(bb[:, 0].astype(np.int32), 0, w)
    y1 = np.clip(bb[:, 1].astype(np.int32), 0, h)
    x2 = np.clip(bb[:, 2].astype(np.int32), 0, w)
    y2 = np.clip(bb[:, 3].astype(np.int32), 0, h)
    cnt = np.maximum(y2 - y1, 0) * np.maximum(x2 - x1, 0)
    x2e = np.maximum(x2, x1)
    y2e = np.maximum(y2, y1)

    R = CAh[y2e] - CAh[y1]    # [300,42] f32
    C = CBw[x2e] - CBw[x1]    # [300,42] f32
    qi = query_idx[n].astype(np.int64)
    matched = np.zeros(Q, bool)
    matched[qi] = True
    nm_valid = (cnt > 0) & (~matched)
    inv = np.zeros(Q, np.float32)
    inv[nm_valid] = (np.float32(1.0)
                     / np.maximum(cnt, 1).astype(np.float32)[nm_valid])
    ovec = np.where(nm_valid, np.float32(0.0),
                    np.float32(NEG)).astype(np.float32)

    # ct3m [126, 3, Q]: C^T replicated per 42-row block s, zero-masked so
    # matmul s contracts ONLY block s of f2 (everything stays partition-0
    # aligned -- no matmul tile_position offsets)
    ct3m = np.zeros((126, 3, Q), np.float32)
    for s in range(3):
        ct3m[42 * s:42 * s + 42, s, :] = C.T
    # r2t3 [14, 3, Q]: R-weight for PSUM row-block layout (f2 column k,
    # block s <-> y = 3k + s), with per-query 1/cnt and the 1/2048
    # channel-mean scale folded in
    r2t3 = np.zeros((14, 3, Q), np.float32)
    rt = R.T * (inv[None, :] * np.float32(1.0 / CF))  # [42, Q]
    for s in range(3):
        r2t3[:, s, :] = rt[[3 * k + s for k in range(14)], :]

    ti = tgt_idx[n].astype(np.int64)
    tcls = tgt_labels[n][ti].astype(np.int64)      # [20]
    Wm = np.zeros((QP, NUM_CLASSES), np.float32)
    np.add.at(Wm, (qi, tcls), np.float32(1.0))
    qcnt = np.zeros(QP, np.float32)
    np.add.at(qcnt, qi, np.float32(1.0))
    wsum = Wm.sum(1)
    valid300 = np.zeros(QP, np.float32)
    valid300[:Q] = 1.0
    matched_bin = np.zeros(QP, np.float32)
    matched_bin[:Q][matched] = 1.0
    # fold -2/M into the matched-CE weights; pmb cols:
    # 0: qcnt * (-2/M)   (bce_matched coefficient)
    # 1: wsum * (-2/M)   (matched-CE logZ coefficient)
    # 2: rest0 = valid - matched  (rest mask before top-5 subtraction)
    sm = np.float32(-2.0 / M)
    Wm *= sm
    pmb = np.ascontiguousarray(
        np.stack([qcnt * sm, wsum * sm, valid300 - matched_bin,
                  np.zeros(QP, np.float32)], -1))  # [384,4]

    qselt = np.zeros((QP, M), np.float32)
    qselt[qi, np.arange(M)] = 1.0
    pbpm = np.zeros((QP, 4), np.float32)
    pbpm[:Q] = pb
    lg = np.zeros((QP, CC), np.float32)
    lg[:Q] = pred_logits[n].astype(np.float32)

    tb = (tgt_boxes[n][ti].astype(np.float32) / scale).astype(np.float32)
    txyxy = np.stack([tb[:, 0] - tb[:, 2] / 2, tb[:, 1] - tb[:, 3] / 2,
                      tb[:, 0] + tb[:, 2] / 2, tb[:, 1] + tb[:, 3] / 2], -1)
    areat = ((txyxy[:, 2] - txyxy[:, 0])
             * (txyxy[:, 3] - txyxy[:, 1])).reshape(M, 1)

    # pack the per-query tensors into one [384, 211] array (fewer DMAs):
    # cols 0:92 logits | 92:183 W | 183:187 pmb | 187:207 qsel^T | 207:211 boxes
    big = np.zeros((QP, 211), np.float32)
    big[:, 0:CC] = lg
    big[:, CC:CC + NUM_CLASSES] = Wm
    big[:, 183:187] = pmb
    big[:, 187:207] = qselt
    big[:, 207:211] = pbpm
    # pack20: tx | area_t | tgt_bb ; pack1: ovec
    p20 = np.zeros((M, 9), np.float32)
    p20[:, 0:4] = txyxy
    p20[:, 4:5] = areat
    p20[:, 5:9] = tb
    p1 = np.zeros((1, 304), np.float32)
    p1[0, 0:Q] = ovec
    return dict(ct3m=np.ascontiguousarray(
                    ct3m.reshape(126, 3 * Q)).astype(ml_dtypes.bfloat16),
                r2t3=np.ascontiguousarray(r2t3.reshape(14, 3 * Q)),
                big=np.ascontiguousarray(big),
                p20=np.ascontiguousarray(p20), p1=p1)


# ------------------------------------------------------------- device build

DEBUG_OUTS = False


def _build_nc():
    nc = bass.Bass()
    feat = nc.dram_tensor("feat", [CF, POS], F32, kind="ExternalInput")
    ct3m = nc.dram_tensor("ct3m", [126, 3 * Q], BF16, kind="ExternalInput")
    r2t3 = nc.dram_tensor("r2t3", [14, 3 * Q], F32, kind="ExternalInput")
    big = nc.dram_tensor("big", [QP, 211], F32, kind="ExternalInput")
    p20 = nc.dram_tensor("p20", [M, 9], F32, kind="ExternalInput")
    p1 = nc.dram_tensor("p1", [1, 304], F32, kind="ExternalInput")
    loss = nc.dram_tensor("loss", [1, 1], F32, kind="ExternalOutput")
    if DEBUG_OUTS:
        dbg_means = nc.dram_tensor("dbg_means", [1, Q], F32,
                                   kind="ExternalOutput")
        dbg_tkf = nc.dram_tensor("dbg_tkf", [1, QP], F32,
                                 kind="ExternalOutput")
        dbg_f2s = None

    with TileContext(nc) as tc:
        with (
            tc.tile_pool(name="feat", bufs=12) as fp,
            tc.tile_pool(name="featb", bufs=2) as fbp,
            tc.tile_pool(name="cst", bufs=1) as cp,
            tc.tile_pool(name="wrk", bufs=1) as wp,
            tc.tile_pool(name="ps_f2", bufs=1, space="PSUM") as pp_f2,
            tc.tile_pool(name="ps_a", bufs=1, space="PSUM") as pp_a,
            tc.tile_pool(name="ps_b", bufs=1, space="PSUM") as pp_b,
            tc.tile_pool(name="ps_tt", bufs=1, space="PSUM") as pp_tt,
        ):
            # one shared small-PSUM tile (sub-bank matmul outputs):
            # cols 0:4 q_ps | 4:6 s_ps | 6:12 xp | 12:15 tk_ps
            psA = pp_a.tile([128, 16], F32)
            # stream units: 7 pairs (fused DVE add+cast -> bf16) + 2 singles
            # (cast-only, slivered per chunk) so the tail chases the last
            # bytes with the shortest possible chain
            pairs = [(2 * t, 2 * t + 1) for t in range(7)]
            ftiles = {}

            def fetch_tile(tt):
                if tt in ftiles:
                    return
                ft = fp.tile([128, POS], F32, tag="feat")
                nc.sync.dma_start(ft[:], feat[128 * tt:128 * (tt + 1), :])
                ftiles[tt] = ft

            def fetch_single(tt):
                # per-chunk sliver DMAs so the tail chases arrivals
                ft = fp.tile([128, POS], F32, tag="feat")
                for lo, hi, _ in CHUNKS:
                    nc.sync.dma_start(ft[:, lo:hi],
                                      feat[128 * tt:128 * (tt + 1), lo:hi])
                ftiles[tt] = ft

            # two feat tiles first (floods the DMA queue), then the small
            # prologue tensors (the prologue matmuls sit ahead of the stream
            # matmuls in the tensor queue, so big_sb must land early too)
            for tt in range(2):
                fetch_tile(tt)
            big_sb = cp.tile([128, 3, 211], F32)
            nc.sync.dma_start(big_sb[:],
                              big[:].rearrange("(t p) c -> p t c", p=128))
            ct3m_sb = cp.tile([126, 3, Q], BF16)
            nc.sync.dma_start(ct3m_sb[:],
                              ct3m[:].rearrange("p (s q) -> p s q", s=3))
            r2t3_sb = cp.tile([14, 3, Q], F32)
            nc.sync.dma_start(r2t3_sb[:],
                              r2t3[:].rearrange("p (s q) -> p s q", s=3))
            p20_sb = cp.tile([M, 9], F32)
            nc.sync.dma_start(p20_sb[:], p20[:])
            p1_sb = cp.tile([1, 304], F32)
            nc.sync.dma_start(p1_sb[:], p1[:])
            for tt in range(2, 8):
                fetch_tile(tt)
            lg_sb = big_sb[:, :, 0:CC]
            w_sb = big_sb[:, :, CC:CC + NUM_CLASSES]
            pmb_sb = big_sb[:, :, 183:187]
            qs_sb = big_sb[:, :, 187:207]
            pb_sb = big_sb[:, :, 207:211]

            # --- per-query softmax / objectness terms ---
            mxl = wp.tile([128, 3], F32)
            nc.vector.tensor_reduce(mxl[:], lg_sb[:, :, 0:NUM_CLASSES],
                                    AX.X, ALU.max)
            negm = wp.tile([128, 3], F32)
            nc.vector.tensor_scalar_mul(negm[:], mxl[:], -1.0)
            e1 = wp.tile([128, 3, NUM_CLASSES], F32)
            se = wp.tile([128, 3], F32)
            for t in range(3):
                nc.scalar.activation(e1[:, t, :], lg_sb[:, t, 0:NUM_CLASSES],
                                     AF.Exp, bias=negm[:, t:t + 1],
                                     accum_out=se[:, t:t + 1])
            rp = wp.tile([128, 3], F32)
            nc.vector.reciprocal(rp[:], se[:])
            p = wp.tile([128, 3, NUM_CLASSES], F32)
            for t in range(3):
                nc.scalar.activation(p[:, t, :], e1[:, t, :], AF.Copy,
                                     scale=rp[:, t:t + 1])
            mx2 = wp.tile([128, 3], F32)
            nc.vector.tensor_reduce(mx2[:], p[:], AX.X, ALU.max)
            negm2 = wp.tile([128, 3], F32)
            nc.vector.tensor_scalar_mul(negm2[:], mx2[:], -1.0)
            e2 = wp.tile([128, 3, NUM_CLASSES], F32)
            s2 = wp.tile([128, 3], F32)
            for t in range(3):
                nc.scalar.activation(e2[:, t, :], p[:, t, :], AF.Exp,
                                     bias=negm2[:, t:t + 1],
                                     accum_out=s2[:, t:t + 1])
            lnz = wp.tile([128, 3], F32)
            nc.scalar.activation(lnz[:], s2[:], AF.Ln)
            off = wp.tile([128, 3], F32)
            nc.vector.tensor_add(off[:], mx2[:], lnz[:])
            # logp90s = (p90 - off) * (-2/TOPK)  (pseudo-CE, coeff folded)
            logp90 = wp.tile([128, 3], F32)
            nc.vector.tensor_sub(logp90[:], p[:, :, NUM_CLASSES - 1], off[:])
            logp90s = wp.tile([128, 3], F32)
            nc.vector.tensor_scalar_mul(logp90s[:], logp90[:], -2.0 / TOPK)
            wpd = wp.tile([128, 3, NUM_CLASSES], F32)
            nc.vector.tensor_mul(wpd[:], w_sb[:], p[:, :, 0:NUM_CLASSES])
            wps = wp.tile([128, 3], F32)
            nc.vector.tensor_reduce(wps[:], wpd[:], AX.X, ALU.add)
            ows = wp.tile([128, 3], F32)
            nc.vector.tensor_mul(ows[:], off[:], pmb_sb[:, :, 1])
            pobj = wp.tile([128, 3], F32)
            nc.scalar.activation(pobj[:], lg_sb[:, :, CC - 1], AF.Sigmoid)
            lnp = wp.tile([128, 3], F32)
            nc.scalar.activation(lnp[:], pobj[:], AF.Ln)
            Lobj = wp.tile([128, 3], F32)
            nc.vector.tensor_single_scalar(Lobj[:], lnp[:], -100.0, ALU.max)
            Lobjs = wp.tile([128, 3], F32)
            nc.vector.tensor_scalar_mul(Lobjs[:], Lobj[:], -2.0 / TOPK)
            u_ = wp.tile([128, 3], F32)
            nc.vector.tensor_scalar(u_[:], pobj[:], -1.0, 1.0,
                                    ALU.mult, ALU.add)
            lnu = wp.tile([128, 3], F32)
            nc.scalar.activation(lnu[:], u_[:], AF.Ln)
            # nl1ms = -max(log1p(-p), -100) * (2/NREST)  (rest-BCE folded)
            nl1ms = wp.tile([128, 3], F32)
            nc.vector.tensor_scalar(nl1ms[:], lnu[:], -100.0, -2.0 / NREST,
                                    ALU.max, ALU.mult)
            # V columns: 0 matched-CE (prologue) | 1 logp90s*tk | 2 Lobjs*tk
            #            | 3 bce_matched (prologue) | 4 nl1ms*rest
            V = wp.tile([128, 3, 5], F32)
            nc.vector.tensor_sub(V[:, :, 0], wps[:], ows[:])
            nc.vector.tensor_mul(V[:, :, 3], Lobj[:], pmb_sb[:, :, 0])

            # --- matched-pair L1 + IoU ---
            q_ps = psA[0:M, 0:4]
            for t in range(3):
                nc.tensor.matmul(q_ps[:], qs_sb[:, t, :], pb_sb[:, t, :],
                                 start=(t == 0), stop=(t == 2))
            qb = wp.tile([M, 4], F32)
            nc.vector.tensor_copy(qb[:], q_ps[:])
            half = wp.tile([M, 2], F32)
            nc.scalar.mul(half[:], qb[:, 2:4], 0.5)
            axy = wp.tile([M, 4], F32)
            nc.vector.tensor_sub(axy[:, 0:2], qb[:, 0:2], half[:])
            nc.vector.tensor_add(axy[:, 2:4], qb[:, 0:2], half[:])
            ixy = wp.tile([M, 4], F32)
            nc.vector.tensor_tensor(ixy[:, 0:2], axy[:, 0:2], p20_sb[:, 0:2],
                                    ALU.max)
            nc.vector.tensor_tensor(ixy[:, 2:4], axy[:, 2:4], p20_sb[:, 2:4],
                                    ALU.min)
            whd = wp.tile([M, 2], F32)
            nc.vector.tensor_sub(whd[:], ixy[:, 2:4], ixy[:, 0:2])
            whc = wp.tile([M, 2], F32)
            nc.vector.tensor_single_scalar(whc[:], whd[:], 0.0, ALU.max)
            inter = wp.tile([M, 1], F32)
            nc.vector.tensor_mul(inter[:], whc[:, 0:1], whc[:, 1:2])
            awh = wp.tile([M, 2], F32)
            nc.vector.tensor_sub(awh[:], axy[:, 2:4], axy[:, 0:2])
            areaa = wp.tile([M, 1], F32)
            nc.vector.tensor_mul(areaa[:], awh[:, 0:1], awh[:, 1:2])
            us = wp.tile([M, 1], F32)
            nc.vector.tensor_add(us[:], areaa[:], p20_sb[:, 4:5])
            us2 = wp.tile([M, 1], F32)
            nc.vector.tensor_sub(us2[:], us[:], inter[:])
            us3 = wp.tile([M, 1], F32)
            nc.vector.tensor_single_scalar(us3[:], us2[:], 1e-9, ALU.add)
            ru = wp.tile([M, 1], F32)
            nc.vector.reciprocal(ru[:], us3[:])
            pk = wp.tile([M, 2], F32)
            nc.vector.tensor_mul(pk[:, 0:1], inter[:], ru[:])
            d = wp.tile([M, 4], F32)
            nc.vector.tensor_sub(d[:], qb[:], p20_sb[:, 5:9])
            dsq = wp.tile([M, 4], F32)
            nc.vector.tensor_mul(dsq[:], d[:], d[:])
            nc.vector.tensor_reduce(pk[:, 1:2], dsq[:], AX.X, ALU.add)
            ones20 = cp.tile([M, 1], F32)
            nc.vector.memset(ones20[:], 1.0)
            s_ps = psA[0:1, 4:6]
            nc.tensor.matmul(s_ps[:], ones20[:], pk[:], start=True, stop=True)
            # base = 2*(M - sum_iou) + 5*sqrt(sum_l1sq)
            l1v = wp.tile([1, 1], F32)
            nc.scalar.activation(l1v[:], s_ps[0:1, 1:2], AF.Sqrt)
            b0 = wp.tile([1, 1], F32)
            nc.vector.tensor_scalar(b0[:], s_ps[0:1, 0:1], -2.0, 2.0 * M,
                                    ALU.mult, ALU.add)
            l15 = wp.tile([1, 1], F32)
            nc.vector.tensor_scalar_mul(l15[:], l1v[:], 5.0)
            base = wp.tile([1, 1], F32)
            nc.vector.tensor_add(base[:], b0[:], l15[:])
            # final-accumulator PSUM: cols 0:5 from the V matmuls, col 5
            # preloaded with base (start=True only zeroes cols 0:5)
            xp = psA[0:1, 6:12]
            nc.vector.tensor_copy(xp[0:1, 5:6], base[:])
            # means PSUM preloaded with the invalid/matched NEG offsets
            b_ps = pp_b.tile([1, Q], F32)
            nc.vector.tensor_copy(b_ps[:], p1_sb[0:1, 0:Q])
            # constants
            ones128b = cp.tile([128, 1], BF16)
            nc.vector.memset(ones128b[:], 1.0)
            ones128f = cp.tile([128, 1], F32)
            nc.vector.memset(ones128f[:], 1.0)
            one1b = cp.tile([1, 1], BF16)
            nc.vector.memset(one1b[:], 1.0)
            ones14b = cp.tile([14, 1], BF16)
            nc.vector.memset(ones14b[:], 1.0)
            tkf = wp.tile([1, QP], BF16)
            nc.vector.memset(tkf[:], 0.0)

            # ===== A: channel sum (memory-bound stream) =====
            # f2 [126, 14] PSUM: column k = channel-sum of positions
            # [126k, 126(k+1));  block-matmuls with the feature tile as the
            # STATIONARY operand land the sums partition-major directly.
            f2 = pp_f2.tile([126, 14], F32)
            # HW probe: interleaved start=True accumulation groups in one
            # PSUM bank zero earlier columns' partial sums. Zero-init once
            # and accumulate with start=False throughout instead.
            nc.vector.memset(f2[:], 0.0)
            nunits = 9

            for ui in range(nunits):
                # prefetch two units ahead
                for ahead in range(ui, min(ui + 3, nunits)):
                    if ahead == 7:
                        fetch_tile(14)
                    elif ahead == 8:
                        if 15 not in ftiles:
                            fetch_single(15)
                    else:
                        for tt in pairs[ahead]:
                            fetch_tile(tt)
                fb = fbp.tile([128, POS], BF16, tag="featb")
                if ui < 7:
                    a, b = pairs[ui]
                    for lo, hi, ks in CHUNKS:
                        nc.vector.tensor_add(fb[:, lo:hi],
                                             ftiles[a][:, lo:hi],
                                             ftiles[b][:, lo:hi])
                        for k in ks:
                            nc.tensor.matmul(
                                f2[:, k:k + 1],
                                fb[:, 126 * k:126 * (k + 1)],
                                ones128b[:],
                                start=False, stop=False,
                                skip_group_check=True)
                else:
                    ft = ftiles[7 + ui]
                    for ci, (lo, hi, ks) in enumerate(CHUNKS):
                        # last chunk's cast on DVE (faster than ACT)
                        if (ci + ui) % 2 == 1:
                            nc.vector.tensor_copy(fb[:, lo:hi],
                                                  ft[:, lo:hi])
                        else:
                            nc.scalar.copy(fb[:, lo:hi], ft[:, lo:hi])
                        for k in ks:
                            nc.tensor.matmul(
                                f2[:, k:k + 1],
                                fb[:, 126 * k:126 * (k + 1)],
                                ones128b[:],
                                start=False, stop=(ui == nunits - 1),
                                skip_group_check=True)

            # ===== B: crop sums + means =====
            f2s = wp.tile([126, 14], BF16)
            nc.vector.tensor_copy(f2s[:], f2[:])
            gcbs = []
            for s in range(3):
                tt_ps = pp_tt.tile([14, Q], F32, tag=f"tt{s}")
                nc.tensor.matmul(tt_ps[:], f2s[:], ct3m_sb[:, s, :],
                                 start=True, stop=True)
                g = wp.tile([14, Q], BF16, tag=f"gcb{s}")
                nc.vector.tensor_mul(g[:], tt_ps[:], r2t3_sb[:, s, :])
                gcbs.append(g)
            for s in range(3):
                nc.tensor.matmul(b_ps[:], ones14b[:], gcbs[s][:],
                                 start=False, stop=(s == 2),
                                 skip_group_check=True)
            means = b_ps

            # ===== C: top-5 mask =====
            mx8 = wp.tile([1, 8], F32)
            nc.vector.max(mx8[:], means[:])
            nc.vector.tensor_scalar(tkf[0:1, 0:Q], means[:],
                                    mx8[0:1, TOPK - 1:TOPK], None,
                                    ALU.is_ge)

            # ===== D: mask to partition layout + V products =====
            tk_ps = psA[:, 12:15]
            for t in range(3):
                nc.tensor.matmul(tk_ps[:, t:t + 1],
                                 tkf[0:1, 128 * t:128 * (t + 1)], one1b[:],
                                 start=True, stop=True)
            rest = wp.tile([128, 3], F32)
            nc.vector.tensor_sub(rest[:], pmb_sb[:, :, 2], tk_ps[:])
            nc.vector.tensor_mul(V[:, :, 1], logp90s[:], tk_ps[:])
            nc.vector.tensor_mul(V[:, :, 2], Lobjs[:], tk_ps[:])
            nc.vector.tensor_mul(V[:, :, 4], nl1ms[:], rest[:])
            for t in range(3):
                nc.tensor.matmul(xp[0:1, 0:5], ones128f[:], V[:, t, :],
                                 start=(t == 0), stop=(t == 2),
                                 skip_group_check=True)

            # ===== G: final sum =====
            lossv = wp.tile([1, 1], F32)
            nc.vector.tensor_reduce(lossv[:], xp[0:1, 0:6], AX.X, ALU.add)
            nc.sync.dma_start(loss[:], lossv[:])
            if DEBUG_OUTS:
                mcopy = wp.tile([1, Q], F32)
                nc.vector.tensor_copy(mcopy[:], b_ps[:])
                nc.sync.dma_start(dbg_means[:], mcopy[:])
                tcopy = wp.tile([1, QP], F32)
                nc.vector.tensor_copy(tcopy[:], tkf[:])
                nc.sync.dma_start(dbg_tkf[:], tcopy[:])
    _split_sync_waits(nc)
    return nc


_NC_CACHE = None


def kernel(img_features, pred_logits, pred_boxes, tgt_labels, tgt_boxes,
           query_idx, tgt_idx, h, w):
    global _NC_CACHE
    h = int(h)
    w = int(w)
    img_features = np.asarray(img_features, np.float32)
    pred_logits = np.asarray(pred_logits, np.float32)
    pred_boxes = np.asarray(pred_boxes, np.float32)
    tgt_labels = np.asarray(tgt_labels)
    tgt_boxes = np.asarray(tgt_boxes, np.float32)
    query_idx = np.asarray(query_idx)
    tgt_idx = np.asarray(tgt_idx)

    CAh = _interp_cummat(h, HF)
    CBw = _interp_cummat(w, WF)

    in_maps = []
    for n in range(N):
        m = _prep_core(n, pred_logits, pred_boxes, tgt_labels, tgt_boxes,
                       query_idx, tgt_idx, h, w, CAh, CBw)
        m["feat"] = np.ascontiguousarray(
            img_features[n].reshape(CF, POS))
        in_maps.append(m)

    if _NC_CACHE is None:
        _NC_CACHE = _build_nc()
    try:
        res = run_bass_kernel_spmd(_NC_CACHE, in_maps,
                                   core_ids=list(range(N)))
    except Exception:
        # transient NRT device errors have been observed on this fabric;
        # one rebuild+retry recovers
        _NC_CACHE = _build_nc()
        res = run_bass_kernel_spmd(_NC_CACHE, in_maps,
                                   core_ids=list(range(N)))
    total = np.float32(0.0)
    for r in res.results:
        total = total + np.float32(r["loss"][0, 0])
    return np.asarray(total, np.float32)


# revision 17
# speedup vs baseline: 1.0108x; 1.0108x over previous
"""Trainium2 Bass kernel for nn_DETRLoss.

Strategy (pure data parallel, batch dim N=8 over 8 NeuronCores):

The only memory-heavy input is img_features [8, 2048, 42, 42] (115.6 MB).
It feeds the loss ONLY through: channel-mean -> bilinear upsample to
(h, w) -> summed-area table -> per-query crop means -> top-5 *indices*.
The SAT of a bilinear upsample evaluated at integer pixel corners is a
bilinear form of the 42x42 channel-mean f:

    sat[y, x] = CA[y] @ f @ CB[x]^T

where CA/CB are cumulative-sum rows of the (analytic) resize matrices.
So each query's crop sum is (CA[y2]-CA[y1]) @ f @ (CB[x2]-CB[x1])^T:
no 1333x1333 upsample or SAT is ever materialized. The crop means feed
ONLY a top-5 selection, so small rounding differences are harmless.

Per core (one image): stream 2048x1764 features (14.45 MB); the channel
sum is computed with the FEATURE TILE AS THE STATIONARY matmul operand
(ones as the moving vector), so each 126-column block lands directly as
one PSUM column of f2 [126, 14] -- partition-major, no [1, 1764] row,
no reshape round-trip. The crop means follow from 3 masked matmuls
(126 = 3x42 row-blocks, mask baked into the host-built C weights), an
elementwise R-weight multiply, and a ones contraction. Top-5 via Max8 +
MatchReplace; all CE/BCE/L1/IoU terms on-chip with host-folded scale
coefficients; the last two tiles stream as 4 column-slivers each so the
tail chases the final DMA bytes. Output: per-image scalar loss; host
sums the 8 scalars.
"""

import ml_dtypes
import numpy as np

import bass_rust
import concourse.bass as bass
import concourse.mybir as mybir
from concourse.bass_utils import run_bass_kernel_spmd
from concourse.tile import TileContext

F32 = mybir.dt.float32
BF16 = mybir.dt.bfloat16
AF = mybir.ActivationFunctionType
ALU = mybir.AluOpType
AX = mybir.AxisListType

N, Q, CC = 8, 300, 92
CF, HF, WF = 2048, 42, 42
M, TOPK = 20, 5
NUM_CLASSES = 91
NEG = -1e11
QP = 384  # Q padded to 3*128
POS = HF * WF  # 1764
NREST = Q - M - TOPK  # 275: matched queries are unique, top-5 disjoint

# column chunks (sliver DMAs / fused-add chunks), aligned to 126-blocks
CHUNKS = [(0, 504, range(0, 4)), (504, 1008, range(4, 8)),
          (1008, 1386, range(8, 11)), (1386, 1764, range(11, 14))]


def _split_sync_waits(nc, max_waits=1):
    """This walrus build rejects >2 sync waits on one instruction ("Too
    many sync wait commands"); hoist extra waits onto same-engine nops
    emitted immediately before the instruction (identical semantics:
    engines process waits in program order)."""
    ctr = 0
    for f in nc.m.functions:
        for bb in f.blocks:
            out = []
            for inst in bb.instructions:
                si = inst.sync_info
                waits = list(si.on_wait) if si and si.on_wait else []
                if len(waits) > max_waits:
                    for w in waits[:-max_waits]:
                        ctr += 1
                        out.append(bass_rust.InstNoOp(
                            name=f"I-wsplit{ctr}", engine=inst.engine,
                            ins=[], outs=[],
                            sync_info=bass_rust.SyncInfo(
                                on_wait=[w], on_update=[])))
                    inst.sync_info = bass_rust.SyncInfo(
                        on_wait=waits[-max_waits:],
                        on_update=list(si.on_update or []))
                out.append(inst)
            bb.instructions = out


# ---------------------------------------------------------------- host prep

def _interp_cummat(out_size, in_size):
    """CA [out_size+1, in_size] with CA[y] = sum_{i<y} A[i,:], A the
    half-pixel-centered bilinear resize matrix (jax.image.resize)."""
    A = np.zeros((out_size, in_size), np.float64)
    scale = in_size / out_size
    for i in range(out_size):
        src = (i + 0.5) * scale - 0.5
        i0 = int(np.floor(src))
        w1 = src - i0
        j0 = min(max(i0, 0), in_size - 1)
        j1 = min(max(i0 + 1, 0), in_size - 1)
        A[i, j0] += 1.0 - w1
        A[i, j1] += w1
    CA = np.zeros((out_size + 1, in_size), np.float64)
    np.cumsum(A, 0, out=CA[1:])
    return CA.astype(np.float32)


def _prep_core(n, pred_logits, pred_boxes, tgt_labels, tgt_boxes,
               query_idx, tgt_idx, h, w, CAh, CBw):
    """Build the small per-core input tensors (everything except feat)."""
    scale = np.array([w, h, w, h], np.float32)
    pb = pred_boxes[n].astype(np.float32)  # [300,4]
    cx, cy, bw, bh = pb[:, 0], pb[:, 1], pb[:, 2], pb[:, 3]
    xy = np.stack([cx - bw / 2, cy - bh / 2, cx + bw / 2, cy + bh / 2], -1)
    bb = xy * scale
    x1 = np.clip(bb[:, 0].astype(np.int32), 0, w)
    y1 = np.clip(bb[:, 1].astype(np.int32), 0, h)
    x2 = np.clip(bb[:, 2].astype(np.int32), 0, w)
    y2 = np.clip(bb[:, 3].astype(np.int32), 0, h)
    cnt = np.maximum(y2 - y1, 0) * np.maximum(x2 - x1, 0)
    x2e = np.maximum(x2, x1)
    y2e = np.maximum(y2, y1)

    R = CAh[y2e] - CAh[y1]    # [300,42] f32
    C = CBw[x2e] - CBw[x1]    # [300,42] f32
    qi = query_idx[n].astype(np.int64)
    matched = np.zeros(Q, bool)
    matched[qi] = True
    nm_valid = (cnt > 0) & (~matched)
    inv = np.zeros(Q, np.float32)
    inv[nm_valid] = (np.float32(1.0)
                     / np.maximum(cnt, 1).astype(np.float32)[nm_valid])
    ovec = np.where(nm_valid, np.float32(0.0),
                    np.float32(NEG)).astype(np.float32)

    # ct3m [126, 3, Q]: C^T replicated per 42-row block s, zero-masked so
    # matmul s contracts ONLY block s of f2 (everything stays partition-0
    # aligned -- no matmul tile_position offsets)
    ct3m = np.zeros((126, 3, Q), np.float32)
    for s in range(3):
        ct3m[42 * s:42 * s + 42, s, :] = C.T
    # r2t3 [14, 3, Q]: R-weight for PSUM row-block layout (f2 column k,
    # block s <-> y = 3k + s), with per-query 1/cnt and the 1/2048
    # channel-mean scale folded in
    r2t3 = np.zeros((14, 3, Q), np.float32)
    rt = R.T * (inv[None, :] * np.float32(1.0 / CF))  # [42, Q]
    for s in range(3):
        r2t3[:, s, :] = rt[[3 * k + s for k in range(14)], :]

    ti = tgt_idx[n].astype(np.int64)
    tcls = tgt_labels[n][ti].astype(np.int64)      # [20]
    Wm = np.zeros((QP, NUM_CLASSES), np.float32)
    np.add.at(Wm, (qi, tcls), np.float32(1.0))
    qcnt = np.zeros(QP, np.float32)
    np.add.at(qcnt, qi, np.float32(1.0))
    wsum = Wm.sum(1)
    valid300 = np.zeros(QP, np.float32)
    valid300[:Q] = 1.0
    matched_bin = np.zeros(QP, np.float32)
    matched_bin[:Q][matched] = 1.0
    # fold -2/M into the matched-CE weights; pmb cols:
    # 0: qcnt * (-2/M)   (bce_matched coefficient)
    # 1: wsum * (-2/M)   (matched-CE logZ coefficient)
    # 2: rest0 = valid - matched  (rest mask before top-5 subtraction)
    sm = np.float32(-2.0 / M)
    Wm *= sm
    pmb = np.ascontiguousarray(
        np.stack([qcnt * sm, wsum * sm, valid300 - matched_bin,
                  np.zeros(QP, np.float32)], -1))  # [384,4]

    qselt = np.zeros((QP, M), np.float32)
    qselt[qi, np.arange(M)] = 1.0
    pbpm = np.zeros((QP, 4), np.float32)
    pbpm[:Q] = pb
    lg = np.zeros((QP, CC), np.float32)
    lg[:Q] = pred_logits[n].astype(np.float32)

    tb = (tgt_boxes[n][ti].astype(np.float32) / scale).astype(np.float32)
    txyxy = np.stack([tb[:, 0] - tb[:, 2] / 2, tb[:, 1] - tb[:, 3] / 2,
                      tb[:, 0] + tb[:, 2] / 2, tb[:, 1] + tb[:, 3] / 2], -1)
    areat = ((txyxy[:, 2] - txyxy[:, 0])
             * (txyxy[:, 3] - txyxy[:, 1])).reshape(M, 1)

    # pack the per-query tensors into one [384, 211] array (fewer DMAs):
    # cols 0:92 logits | 92:183 W | 183:187 pmb | 187:207 qsel^T | 207:211 boxes
    big = np.zeros((QP, 211), np.float32)
    big[:, 0:CC] = lg
    big[:, CC:CC + NUM_CLASSES] = Wm
    big[:, 183:187] = pmb
    big[:, 187:207] = qselt
    big[:, 207:211] = pbpm
    # pack20: tx | area_t | tgt_bb ; pack1: ovec
    p20 = np.zeros((M, 9), np.float32)
    p20[:, 0:4] = txyxy
    p20[:, 4:5] = areat
    p20[:, 5:9] = tb
    p1 = np.zeros((1, 304), np.float32)
    p1[0, 0:Q] = ovec
    return dict(ct3m=np.ascontiguousarray(
                    ct3m.reshape(126, 3 * Q)).astype(ml_dtypes.bfloat16),
                r2t3=np.ascontiguousarray(r2t3.reshape(14, 3 * Q)),
                big=np.ascontiguousarray(big),
                p20=np.ascontiguousarray(p20), p1=p1)


# ------------------------------------------------------------- device build

DEBUG_OUTS = False


def _build_nc():
    nc = bass.Bass()
    feat = nc.dram_tensor("feat", [CF, POS], F32, kind="ExternalInput")
    ct3m = nc.dram_tensor("ct3m", [126, 3 * Q], BF16, kind="ExternalInput")
    r2t3 = nc.dram_tensor("r2t3", [14, 3 * Q], F32, kind="ExternalInput")
    big = nc.dram_tensor("big", [QP, 211], F32, kind="ExternalInput")
    p20 = nc.dram_tensor("p20", [M, 9], F32, kind="ExternalInput")
    p1 = nc.dram_tensor("p1", [1, 304], F32, kind="ExternalInput")
    loss = nc.dram_tensor("loss", [1, 1], F32, kind="ExternalOutput")
    if DEBUG_OUTS:
        dbg_means = nc.dram_tensor("dbg_means", [1, Q], F32,
                                   kind="ExternalOutput")
        dbg_tkf = nc.dram_tensor("dbg_tkf", [1, QP], F32,
                                 kind="ExternalOutput")
        dbg_f2s = None

    with TileContext(nc) as tc:
        with (
            tc.tile_pool(name="feat", bufs=12) as fp,
            tc.tile_pool(name="featb", bufs=2) as fbp,
            tc.tile_pool(name="cst", bufs=1) as cp,
            tc.tile_pool(name="wrk", bufs=1) as wp,
            tc.tile_pool(name="ps_f2", bufs=1, space="PSUM") as pp_f2,
            tc.tile_pool(name="ps_a", bufs=1, space="PSUM") as pp_a,
            tc.tile_pool(name="ps_b", bufs=1, space="PSUM") as pp_b,
            tc.tile_pool(name="ps_tt", bufs=1, space="PSUM") as pp_tt,
        ):
            # one shared small-PSUM tile (sub-bank matmul outputs):
            # cols 0:4 q_ps | 4:6 s_ps | 6:12 xp | 12:15 tk_ps
            psA = pp_a.tile([128, 16], F32)
            # stream units: 7 pairs (fused DVE add+cast -> bf16) + 2 singles
            # (cast-only, slivered per chunk) so the tail chases the last
            # bytes with the shortest possible chain
            pairs = [(2 * t, 2 * t + 1) for t in range(7)]
            ftiles = {}

            def fetch_tile(tt):
                if tt in ftiles:
                    return
                ft = fp.tile([128, POS], F32, tag="feat")
                if tt < 14:
                    nc.sync.dma_start(ft[:], feat[128 * tt:128 * (tt + 1), :])
                ftiles[tt] = ft

            def fetch_single(tt):
                # per-chunk sliver DMAs so the tail chases arrivals
                ft = fp.tile([128, POS], F32, tag="feat")
                for lo, hi, _ in CHUNKS:
                    nc.sync.dma_start(ft[:, lo:hi],
                                      feat[128 * tt:128 * (tt + 1), lo:hi])
                ftiles[tt] = ft

            # two feat tiles first (floods the DMA queue), then the small
            # prologue tensors (the prologue matmuls sit ahead of the stream
            # matmuls in the tensor queue, so big_sb must land early too)
            for tt in range(2):
                fetch_tile(tt)
            big_sb = cp.tile([128, 3, 211], F32)
            nc.sync.dma_start(big_sb[:],
                              big[:].rearrange("(t p) c -> p t c", p=128))
            ct3m_sb = cp.tile([126, 3, Q], BF16)
            nc.sync.dma_start(ct3m_sb[:],
                              ct3m[:].rearrange("p (s q) -> p s q", s=3))
            r2t3_sb = cp.tile([14, 3, Q], F32)
            nc.sync.dma_start(r2t3_sb[:],
                              r2t3[:].rearrange("p (s q) -> p s q", s=3))
            p20_sb = cp.tile([M, 9], F32)
            nc.sync.dma_start(p20_sb[:], p20[:])
            p1_sb = cp.tile([1, 304], F32)
            nc.sync.dma_start(p1_sb[:], p1[:])
            for tt in range(2, 8):
                fetch_tile(tt)
            lg_sb = big_sb[:, :, 0:CC]
            w_sb = big_sb[:, :, CC:CC + NUM_CLASSES]
            pmb_sb = big_sb[:, :, 183:187]
            qs_sb = big_sb[:, :, 187:207]
            pb_sb = big_sb[:, :, 207:211]

            # --- per-query softmax / objectness terms ---
            mxl = wp.tile([128, 3], F32)
            nc.vector.tensor_reduce(mxl[:], lg_sb[:, :, 0:NUM_CLASSES],
                                    AX.X, ALU.max)
            negm = wp.tile([128, 3], F32)
            nc.vector.tensor_scalar_mul(negm[:], mxl[:], -1.0)
            e1 = wp.tile([128, 3, NUM_CLASSES], F32)
            se = wp.tile([128, 3], F32)
            for t in range(3):
                nc.scalar.activation(e1[:, t, :], lg_sb[:, t, 0:NUM_CLASSES],
                                     AF.Exp, bias=negm[:, t:t + 1],
                                     accum_out=se[:, t:t + 1])
            rp = wp.tile([128, 3], F32)
            nc.vector.reciprocal(rp[:], se[:])
            p = wp.tile([128, 3, NUM_CLASSES], F32)
            for t in range(3):
                nc.scalar.activation(p[:, t, :], e1[:, t, :], AF.Copy,
                                     scale=rp[:, t:t + 1])
            mx2 = wp.tile([128, 3], F32)
            nc.vector.tensor_reduce(mx2[:], p[:], AX.X, ALU.max)
            negm2 = wp.tile([128, 3], F32)
            nc.vector.tensor_scalar_mul(negm2[:], mx2[:], -1.0)
            e2 = wp.tile([128, 3, NUM_CLASSES], F32)
            s2 = wp.tile([128, 3], F32)
            for t in range(3):
                nc.scalar.activation(e2[:, t, :], p[:, t, :], AF.Exp,
                                     bias=negm2[:, t:t + 1],
                                     accum_out=s2[:, t:t + 1])
            lnz = wp.tile([128, 3], F32)
            nc.scalar.activation(lnz[:], s2[:], AF.Ln)
            off = wp.tile([128, 3], F32)
            nc.vector.tensor_add(off[:], mx2[:], lnz[:])
            # logp90s = (p90 - off) * (-2/TOPK)  (pseudo-CE, coeff folded)
            logp90 = wp.tile([128, 3], F32)
            nc.vector.tensor_sub(logp90[:], p[:, :, NUM_CLASSES - 1], off[:])
            logp90s = wp.tile([128, 3], F32)
            nc.vector.tensor_scalar_mul(logp90s[:], logp90[:], -2.0 / TOPK)
            wpd = wp.tile([128, 3, NUM_CLASSES], F32)
            nc.vector.tensor_mul(wpd[:], w_sb[:], p[:, :, 0:NUM_CLASSES])
            wps = wp.tile([128, 3], F32)
            nc.vector.tensor_reduce(wps[:], wpd[:], AX.X, ALU.add)
            ows = wp.tile([128, 3], F32)
            nc.vector.tensor_mul(ows[:], off[:], pmb_sb[:, :, 1])
            pobj = wp.tile([128, 3], F32)
            nc.scalar.activation(pobj[:], lg_sb[:, :, CC - 1], AF.Sigmoid)
            lnp = wp.tile([128, 3], F32)
            nc.scalar.activation(lnp[:], pobj[:], AF.Ln)
            Lobj = wp.tile([128, 3], F32)
            nc.vector.tensor_single_scalar(Lobj[:], lnp[:], -100.0, ALU.max)
            Lobjs = wp.tile([128, 3], F32)
            nc.vector.tensor_scalar_mul(Lobjs[:], Lobj[:], -2.0 / TOPK)
            u_ = wp.tile([128, 3], F32)
            nc.vector.tensor_scalar(u_[:], pobj[:], -1.0, 1.0,
                                    ALU.mult, ALU.add)
            lnu = wp.tile([128, 3], F32)
            nc.scalar.activation(lnu[:], u_[:], AF.Ln)
            # nl1ms = -max(log1p(-p), -100) * (2/NREST)  (rest-BCE folded)
            nl1ms = wp.tile([128, 3], F32)
            nc.vector.tensor_scalar(nl1ms[:], lnu[:], -100.0, -2.0 / NREST,
                                    ALU.max, ALU.mult)
            # V columns: 0 matched-CE (prologue) | 1 logp90s*tk | 2 Lobjs*tk
            #            | 3 bce_matched (prologue) | 4 nl1ms*rest
            V = wp.tile([128, 3, 5], F32)
            nc.vector.tensor_sub(V[:, :, 0], wps[:], ows[:])
            nc.vector.tensor_mul(V[:, :, 3], Lobj[:], pmb_sb[:, :, 0])

            # --- matched-pair L1 + IoU ---
            q_ps = psA[0:M, 0:4]
            for t in range(3):
                nc.tensor.matmul(q_ps[:], qs_sb[:, t, :], pb_sb[:, t, :],
                                 start=(t == 0), stop=(t == 2))
            qb = wp.tile([M, 4], F32)
            nc.vector.tensor_copy(qb[:], q_ps[:])
            half = wp.tile([M, 2], F32)
            nc.scalar.mul(half[:], qb[:, 2:4], 0.5)
            axy = wp.tile([M, 4], F32)
            nc.vector.tensor_sub(axy[:, 0:2], qb[:, 0:2], half[:])
            nc.vector.tensor_add(axy[:, 2:4], qb[:, 0:2], half[:])
            ixy = wp.tile([M, 4], F32)
            nc.vector.tensor_tensor(ixy[:, 0:2], axy[:, 0:2], p20_sb[:, 0:2],
                                    ALU.max)
            nc.vector.tensor_tensor(ixy[:, 2:4], axy[:, 2:4], p20_sb[:, 2:4],
                                    ALU.min)
            whd = wp.tile([M, 2], F32)
            nc.vector.tensor_sub(whd[:], ixy[:, 2:4], ixy[:, 0:2])
            whc = wp.tile([M, 2], F32)
            nc.vector.tensor_single_scalar(whc[:], whd[:], 0.0, ALU.max)
            inter = wp.tile([M, 1], F32)
            nc.vector.tensor_mul(inter[:], whc[:, 0:1], whc[:, 1:2])
            awh = wp.tile([M, 2], F32)
            nc.vector.tensor_sub(awh[:], axy[:, 2:4], axy[:, 0:2])
            areaa = wp.tile([M, 1], F32)
            nc.vector.tensor_mul(areaa[:], awh[:, 0:1], awh[:, 1:2])
            us = wp.tile([M, 1], F32)
            nc.vector.tensor_add(us[:], areaa[:], p20_sb[:, 4:5])
            us2 = wp.tile([M, 1], F32)
            nc.vector.tensor_sub(us2[:], us[:], inter[:])
            us3 = wp.tile([M, 1], F32)
            nc.vector.tensor_single_scalar(us3[:], us2[:], 1e-9, ALU.add)
            ru = wp.tile([M, 1], F32)
            nc.vector.reciprocal(ru[:], us3[:])
            pk = wp.tile([M, 2], F32)
            nc.vector.tensor_mul(pk[:, 0:1], inter[:], ru[:])
            d = wp.tile([M, 4], F32)
            nc.vector.tensor_sub(d[:], qb[:], p20_sb[:, 5:9])
            dsq = wp.tile([M, 4], F32)
            nc.vector.tensor_mul(dsq[:], d[:], d[:])
            nc.vector.tensor_reduce(pk[:, 1:2], dsq[:], AX.X, ALU.add)
            ones20 = cp.tile([M, 1], F32)
            nc.vector.memset(ones20[:], 1.0)
            s_ps = psA[0:1, 4:6]
            nc.tensor.matmul(s_ps[:], ones20[:], pk[:], start=True, stop=True)
            # base = 2*(M - sum_iou) + 5*sqrt(sum_l1sq)
            l1v = wp.tile([1, 1], F32)
            nc.scalar.activation(l1v[:], s_ps[0:1, 1:2], AF.Sqrt)
            b0 = wp.tile([1, 1], F32)
            nc.vector.tensor_scalar(b0[:], s_ps[0:1, 0:1], -2.0, 2.0 * M,
                                    ALU.mult, ALU.add)
            l15 = wp.tile([1, 1], F32)
            nc.vector.tensor_scalar_mul(l15[:], l1v[:], 5.0)
            base = wp.tile([1, 1], F32)
            nc.vector.tensor_add(base[:], b0[:], l15[:])
            # final-accumulator PSUM: cols 0:5 from the V matmuls, col 5
            # preloaded with base (start=True only zeroes cols 0:5)
            xp = psA[0:1, 6:12]
            nc.vector.tensor_copy(xp[0:1, 5:6], base[:])
            # means PSUM preloaded with the invalid/matched NEG offsets
            b_ps = pp_b.tile([1, Q], F32)
            nc.vector.tensor_copy(b_ps[:], p1_sb[0:1, 0:Q])
            # constants
            ones128b = cp.tile([128, 1], BF16)
            nc.vector.memset(ones128b[:], 1.0)
            ones128f = cp.tile([128, 1], F32)
            nc.vector.memset(ones128f[:], 1.0)
            one1b = cp.tile([1, 1], BF16)
            nc.vector.memset(one1b[:], 1.0)
            ones14b = cp.tile([14, 1], BF16)
            nc.vector.memset(ones14b[:], 1.0)
            tkf = wp.tile([1, QP], BF16)
            nc.vector.memset(tkf[:], 0.0)

            # ===== A: channel sum (memory-bound stream) =====
            # f2 [126, 14] PSUM: column k = channel-sum of positions
            # [126k, 126(k+1));  block-matmuls with the feature tile as the
            # STATIONARY operand land the sums partition-major directly.
            f2 = pp_f2.tile([126, 14], F32)
            # HW probe: interleaved start=True accumulation groups in one
            # PSUM bank zero earlier columns' partial sums. Zero-init once
            # and accumulate with start=False throughout instead.
            nc.vector.memset(f2[:], 0.0)
            nunits = 9

            for ui in range(nunits):
                # prefetch two units ahead
                for ahead in range(ui, min(ui + 3, nunits)):
                    if ahead >= 7:
                        if 7 + ahead not in ftiles:
                            fetch_single(7 + ahead)
                    else:
                        for tt in pairs[ahead]:
                            fetch_tile(tt)
                fb = fbp.tile([128, POS], BF16, tag="featb")
                if ui < 7:
                    a, b = pairs[ui]
                    for lo, hi, ks in CHUNKS:
                        nc.vector.tensor_add(fb[:, lo:hi],
                                             ftiles[a][:, lo:hi],
                                             ftiles[b][:, lo:hi])
                        for k in ks:
                            nc.tensor.matmul(
                                f2[:, k:k + 1],
                                fb[:, 126 * k:126 * (k + 1)],
                                ones128b[:],
                                start=False, stop=False,
                                skip_group_check=True)
                else:
                    ft = ftiles[7 + ui]
                    for ci, (lo, hi, ks) in enumerate(CHUNKS):
                        # last chunk's cast on DVE (faster than ACT)
                        if (ci + ui) % 2 == 1:
                            nc.vector.tensor_copy(fb[:, lo:hi],
                                                  ft[:, lo:hi])
                        else:
                            nc.scalar.copy(fb[:, lo:hi], ft[:, lo:hi])
                        for k in ks:
                            nc.tensor.matmul(
                                f2[:, k:k + 1],
                                fb[:, 126 * k:126 * (k + 1)],
                                ones128b[:],
                                start=False, stop=(ui == nunits - 1),
                                skip_group_check=True)

            # ===== B: crop sums + means =====
            f2s = wp.tile([126, 14], BF16)
            nc.vector.tensor_copy(f2s[:], f2[:])
            gcbs = []
            for s in range(3):
                tt_ps = pp_tt.tile([14, Q], F32, tag=f"tt{s}")
                nc.tensor.matmul(tt_ps[:], f2s[:], ct3m_sb[:, s, :],
                                 start=True, stop=True)
                g = wp.tile([14, Q], BF16, tag=f"gcb{s}")
                nc.vector.tensor_mul(g[:], tt_ps[:], r2t3_sb[:, s, :])
                gcbs.append(g)
            for s in range(3):
                nc.tensor.matmul(b_ps[:], ones14b[:], gcbs[s][:],
                                 start=False, stop=(s == 2),
                                 skip_group_check=True)
            means = b_ps

            # ===== C: top-5 mask =====
            mx8 = wp.tile([1, 8], F32)
            nc.vector.max(mx8[:], means[:])
            nc.vector.tensor_scalar(tkf[0:1, 0:Q], means[:],
                                    mx8[0:1, TOPK - 1:TOPK], None,
                                    ALU.is_ge)

            # ===== D: mask to partition layout + V products =====
            tk_ps = psA[:, 12:15]
            for t in range(3):
                nc.tensor.matmul(tk_ps[:, t:t + 1],
                                 tkf[0:1, 128 * t:128 * (t + 1)], one1b[:],
                                 start=True, stop=True)
            rest = wp.tile([128, 3], F32)
            nc.vector.tensor_sub(rest[:], pmb_sb[:, :, 2], tk_ps[:])
            nc.vector.tensor_mul(V[:, :, 1], logp90s[:], tk_ps[:])
            nc.vector.tensor_mul(V[:, :, 2], Lobjs[:], tk_ps[:])
            nc.vector.tensor_mul(V[:, :, 4], nl1ms[:], rest[:])
            for t in range(3):
                nc.tensor.matmul(xp[0:1, 0:5], ones128f[:], V[:, t, :],
                                 start=(t == 0), stop=(t == 2),
                                 skip_group_check=True)

            # ===== G: final sum =====
            lossv = wp.tile([1, 1], F32)
            nc.vector.tensor_reduce(lossv[:], xp[0:1, 0:6], AX.X, ALU.add)
            nc.sync.dma_start(loss[:], lossv[:])
            if DEBUG_OUTS:
                mcopy = wp.tile([1, Q], F32)
                nc.vector.tensor_copy(mcopy[:], b_ps[:])
                nc.sync.dma_start(dbg_means[:], mcopy[:])
                tcopy = wp.tile([1, QP], F32)
                nc.vector.tensor_copy(tcopy[:], tkf[:])
                nc.sync.dma_start(dbg_tkf[:], tcopy[:])
    _split_sync_waits(nc)
    return nc


_NC_CACHE = None


def kernel(img_features, pred_logits, pred_boxes, tgt_labels, tgt_boxes,
           query_idx, tgt_idx, h, w):
    global _NC_CACHE
    h = int(h)
    w = int(w)
    img_features = np.asarray(img_features, np.float32)
    pred_logits = np.asarray(pred_logits, np.float32)
    pred_boxes = np.asarray(pred_boxes, np.float32)
    tgt_labels = np.asarray(tgt_labels)
    tgt_boxes = np.asarray(tgt_boxes, np.float32)
    query_idx = np.asarray(query_idx)
    tgt_idx = np.asarray(tgt_idx)

    CAh = _interp_cummat(h, HF)
    CBw = _interp_cummat(w, WF)

    in_maps = []
    for n in range(N):
        m = _prep_core(n, pred_logits, pred_boxes, tgt_labels, tgt_boxes,
                       query_idx, tgt_idx, h, w, CAh, CBw)
        m["feat"] = np.ascontiguousarray(
            img_features[n].reshape(CF, POS))
        in_maps.append(m)

    if _NC_CACHE is None:
        _NC_CACHE = _build_nc()
    try:
        res = run_bass_kernel_spmd(_NC_CACHE, in_maps,
                                   core_ids=list(range(N)))
    except Exception:
        # transient NRT device errors have been observed on this fabric;
        # one rebuild+retry recovers
        _NC_CACHE = _build_nc()
        res = run_bass_kernel_spmd(_NC_CACHE, in_maps,
                                   core_ids=list(range(N)))
    total = np.float32(0.0)
    for r in res.results:
        total = total + np.float32(r["loss"][0, 0])
    return np.asarray(total, np.float32)
